# revision 38
# baseline (speedup 1.0000x reference)
"""Trainium2 Bass kernel for one DNC memory-addressing timestep.

Contract: kernel(**inputs) takes the FULL (unsharded) numpy inputs of
reference.setup_inputs() and returns the full outputs
(read_weights, write_weights, usage, link, precedence) as numpy float32.

Sharding: pure data parallel over batch dim 0 across 8 NeuronCores
(8 batch elements per core, no cross-core communication).

Algorithm notes (per batch element, N=1024 slots, W=64 word, R=4 reads):
  - usage / precedence / write weights: exact elementwise math, batched
    across the 8 per-core elements on partitions 0..7 ("row layout"
    [8, 1024]) so every vector op costs one instruction per core.
  - allocation: the reference sorts usage; here we use the equivalent
    closed form alloc[i] = nonusage[i] * prod_{j: u_j < u_i} u_j
                        = nonusage[i] * exp(sum_j [u_j < u_i] * log u_j)
    computed with comparison tiles (DVE is_gt) contracted against
    log(u) columns on the TensorEngine.  (fp32 ties have ~0 probability
    and were validated against the reference to 3e-8 absmax.)
  - cosine content addressing: dot products via PE after on-chip
    transposes of memory; softmax batched over all 8*5 head-rows.
  - link update (the memory-bound bulk: 8MB/elt of HBM traffic) is two
    fused scalar_tensor_tensor ops per [128, 1024] tile:
        out = (a_i - w_j) * L         (a = 1 - w, per-partition scalar)
        out = (p_j * w_i) + out
    plus a [128,128] diagonal mask multiply, split DVE/GPSIMD.
"""

import os
import sys
from contextlib import ExitStack

for _p in ("/opt/trn_rl_repo", "/root/.axon_site/_ro/trn_rl_repo"):
    if os.path.isdir(_p) and _p not in sys.path:
        sys.path.append(_p)

import numpy as np

import concourse.bass as bass
import concourse.bacc as bacc
import concourse.mybir as mybir
import concourse.tile as tile
from concourse.bass_utils import run_bass_kernel_spmd

F32 = mybir.dt.float32
F32R = mybir.dt.float32r
BF16 = mybir.dt.bfloat16
AF = mybir.ActivationFunctionType
OP = mybir.AluOpType
AX = mybir.AxisListType
ts = bass.ts

EPS = 1e-5
B, N, W, R, NW = 64, 1024, 64, 4, 1
NCORES = 8
E = B // NCORES          # batch elements per core = 8
T = N // 128             # 128-row chunks per link matrix = 8
H = R + NW               # stacked heads (4 read + 1 write) = 5

# How many of the 8 link-update "first op" tiles per element run on
# GPSIMD instead of DVE (load balancing knob).
GPS_STT = int(os.environ.get("K_GPS_STT", "0"))
# Engine for the diagonal mask multiply: "pool" or "dve".
DIAG_ENG = os.environ.get("K_DIAG_ENG", "pool")

TRACE = False
TRACE_KW = {}
LAST_RESULTS = None

_NC = None


def _emit(nc, tc, io):
    (mem_d, rk_d, rs_d, wk_d, ws_d, fg_d, ag_d, wg_d, prw_d, pww_d, pu_d,
     pl_d, pp_d, ident_d, dmask_d, sel8_d, selE4_d, selM_d,
     orw_d, oww_d, ou_d, ol_d, op_d) = io

    ctx = ExitStack()
    cpool = ctx.enter_context(tc.tile_pool(name="consts", bufs=1))
    gpool = ctx.enter_context(tc.tile_pool(name="glob", bufs=1))
    tpool = ctx.enter_context(tc.tile_pool(name="tmp", bufs=5))
    epool = ctx.enter_context(tc.tile_pool(name="pere", bufs=2))
    e1pool = ctx.enter_context(tc.tile_pool(name="pere1", bufs=1))
    erow = ctx.enter_context(tc.tile_pool(name="erow", bufs=10))
    ctpool = ctx.enter_context(tc.tile_pool(name="ct", bufs=3))
    lpool = ctx.enter_context(tc.tile_pool(name="lin", bufs=8))
    opool = ctx.enter_context(tc.tile_pool(name="lout", bufs=4))
    ps_big = ctx.enter_context(tc.tile_pool(name="ps_big", bufs=1, space="PSUM"))
    ps_acc = ctx.enter_context(tc.tile_pool(name="ps_acc", bufs=4, space="PSUM"))
    ps_tr = ctx.enter_context(tc.tile_pool(name="ps_tr", bufs=2, space="PSUM"))

    # ---- constants ----
    ident = cpool.tile([128, 128], F32, tag="ident")
    selE4 = cpool.tile([32, 8], F32, tag="selE4")
    selM = cpool.tile([8, 40], F32, tag="selM")
    epsv = cpool.tile([128, 1], F32, tag="epsv")
    ones_row = cpool.tile([1, 128], F32, tag="ones_row")
    half2 = cpool.tile([2, 128], F32, tag="half2")
    ones3 = cpool.tile([3, 1], F32, tag="ones3")
    dmask = cpool.tile([128, 128], F32, tag="dmask")
    nc.sync.dma_start(dmask[:], dmask_d[:])
    nc.sync.dma_start(ident[:], ident_d[:])
    nc.sync.dma_start(selE4[:], selE4_d[:])
    nc.sync.dma_start(selM[:], selM_d[:])
    nc.vector.memset(epsv[:], EPS)
    nc.vector.memset(ones_row[:], 1.0)
    nc.vector.memset(half2[:], 0.5)
    nc.vector.memset(ones3[:], 1.0)

    # ---- small input loads ----
    prw_all = gpool.tile([32, 1024], F32, tag="prw")       # rows 4e+r
    fg_all = gpool.tile([32, 1], F32, tag="fg")
    pu8 = gpool.tile([8, 1024], F32, tag="pu8")
    pww8 = gpool.tile([8, 1024], F32, tag="pww8")
    K40 = gpool.tile([40, 64], F32, tag="K40")             # rows 5e+h
    st40 = gpool.tile([40, 1], F32, tag="st40")
    nc.sync.dma_start(prw_all[:], prw_d[:, :, :].rearrange("e r n -> (e r) n"))
    nc.sync.dma_start(fg_all[:, 0], fg_d[:, :].rearrange("e r -> (e r)"))
    nc.sync.dma_start(pu8[:], pu_d[:, :])
    nc.sync.dma_start(pww8[:], pww_d[:, 0, :])
    for e in range(E):
        nc.sync.dma_start(K40[e * H:e * H + R, :], rk_d[e, :, :])
        nc.sync.dma_start(K40[e * H + R:e * H + H, :], wk_d[e, :, :])
        nc.sync.dma_start(st40[e * H:e * H + R, 0], rs_d[e, :])
        nc.sync.dma_start(st40[e * H + R:e * H + H, 0], ws_d[e, :])

    # ---- usage (batched rows [8, 1024]) ----
    negfg = gpool.tile([32, 1], F32, tag="negfg")
    nc.scalar.mul(negfg[:], fg_all[:], -1.0)
    t_all = tpool.tile([32, 1024], F32, tag="tmp40", name="t_all")
    nc.scalar.activation(t_all[:], prw_all[:], AF.Identity, bias=1.0, scale=negfg[:])
    lt_all = tpool.tile([32, 1024], F32, tag="tmp40", name="lt_all")
    nc.scalar.activation(lt_all[:], t_all[:], AF.Ln)
    psum_phi = ps_big.tile([128, 1024], F32, tag="pbig", name="psum_phi")
    for h in range(2):
        nc.tensor.matmul(psum_phi[0:8, ts(h, 512)], selE4[:, :], lt_all[:, ts(h, 512)],
                         start=True, stop=True)
    phi8 = tpool.tile([8, 1024], F32, tag="tmp40", name="phi8")
    nc.scalar.activation(phi8[:], psum_phi[0:8, :], AF.Exp)

    onem_pu8 = tpool.tile([8, 1024], F32, tag="tmp40", name="onem_pu8")
    nc.scalar.activation(onem_pu8[:], pu8[:], AF.Identity, bias=1.0, scale=-1.0)
    onem_pww8 = tpool.tile([8, 1024], F32, tag="tmp40", name="onem_pww8")
    nc.scalar.activation(onem_pww8[:], pww8[:], AF.Identity, bias=1.0, scale=-1.0)
    mm2 = tpool.tile([8, 1024], F32, tag="tmp40", name="mm2")
    nc.vector.tensor_mul(mm2[:], onem_pu8[:], onem_pww8[:])
    u18 = tpool.tile([8, 1024], F32, tag="tmp40", name="u18")
    nc.scalar.activation(u18[:], mm2[:], AF.Identity, bias=1.0, scale=-1.0)
    usage8 = tpool.tile([8, 1024], F32, tag="tmp40", name="usage8")
    nc.vector.tensor_mul(usage8[:], u18[:], phi8[:])
    nc.scalar.dma_start(ou_d[:, :], usage8[:])

    ua8 = gpool.tile([8, 1024], F32, tag="ua8")
    nc.scalar.activation(ua8[:], usage8[:], AF.Identity, bias=epsv[0:8, :], scale=1.0 - EPS)
    lua8 = tpool.tile([8, 1024], F32, tag="tmp40", name="lua8")
    nc.scalar.activation(lua8[:], ua8[:], AF.Ln)
    nonu8 = gpool.tile([8, 1024], F32, tag="nonu8")
    nc.scalar.activation(nonu8[:], ua8[:], AF.Identity, bias=1.0, scale=-1.0)

    # column layouts: ucol for compares, lcol -> 3-way bf16 split for matmuls
    ucol = gpool.tile([128, 64], F32, tag="ucol")   # [:, t*8+e]
    lcol = gpool.tile([128, 64], F32, tag="lcol")
    for t in range(T):
        p1 = ps_tr.tile([128, 8], F32, tag="tr", name="p1")
        nc.tensor.transpose(p1[:], ua8[:, ts(t, 128)], ident[0:8, 0:8])
        nc.scalar.copy(ucol[:, ts(t, 8)], p1[:])
        p2 = ps_tr.tile([128, 8], F32, tag="tr", name="p2")
        nc.tensor.transpose(p2[:], lua8[:, ts(t, 128)], ident[0:8, 0:8])
        nc.scalar.copy(lcol[:, ts(t, 8)], p2[:])
    l3 = gpool.tile([128, 192], BF16, tag="l3")
    l3v = l3[:].rearrange("p (c k) -> p c k", k=3)
    hi_f = gpool.tile([128, 64], F32, tag="hi_f")
    r1 = gpool.tile([128, 64], F32, tag="r1")
    nc.scalar.copy(l3v[:, :, 0], lcol[:])
    nc.scalar.copy(hi_f[:], l3v[:, :, 0])
    nc.vector.tensor_sub(r1[:], lcol[:], hi_f[:])
    nc.scalar.copy(l3v[:, :, 1], r1[:])
    nc.scalar.copy(hi_f[:], l3v[:, :, 1])
    nc.vector.tensor_sub(r1[:], r1[:], hi_f[:])
    nc.scalar.copy(l3v[:, :, 2], r1[:])

    # ---- cosine content weights (independent of usage; fully batched) ----
    p_k = ps_tr.tile([64, 40], F32, tag="tr", name="p_k")
    nc.tensor.transpose(p_k[:], K40[:], ident[0:40, 0:40])
    k5t = gpool.tile([64, 40], F32, tag="k5t")
    nc.scalar.copy(k5t[:], p_k[:])
    sqk = gpool.tile([40, 64], F32, tag="sqk")
    nc.scalar.activation(sqk[:], K40[:], AF.Square)
    kn2 = gpool.tile([40, 1], F32, tag="kn2")
    nc.vector.reduce_sum(kn2[:], sqk[:], axis=AX.X)
    kn40 = gpool.tile([40, 1], F32, tag="kn40")
    nc.scalar.activation(kn40[:], kn2[:], AF.Sqrt, bias=epsv[0:40, :])
    e40 = gpool.tile([40, 1], F32, tag="e40")
    nc.scalar.activation(e40[:], st40[:], AF.Exp)
    sp40 = gpool.tile([40, 1], F32, tag="sp40")
    nc.scalar.activation(sp40[:], e40[:], AF.Ln, bias=1.0)

    mncol = gpool.tile([128, 64], F32, tag="mncol")   # [:, t*8+e] = ||mem row||^2
    d40 = gpool.tile([40, 1024], F32, tag="d40")
    for e in range(E):
        mem_e = epool.tile([128, 512], F32, tag="mem_e")
        nc.sync.dma_start(
            mem_e[:].rearrange("p (t w) -> p t w", w=W),
            mem_d[e].rearrange("(t p) w -> p t w", p=128),
        )
        memt = e1pool.tile([64, 1024], F32, tag="memt")
        sqd = epool.tile([128, 64], F32, tag="sqd")
        for t in range(T):
            p3 = ps_tr.tile([64, 128], F32, tag="tr", name="p3")
            nc.tensor.transpose(p3[:], mem_e[:, ts(t, W)], ident[:, :])
            nc.scalar.copy(memt[:, ts(t, 128)], p3[:])
            nc.scalar.activation(sqd[:], mem_e[:, ts(t, W)], AF.Square,
                                 accum_out=mncol[:, t * 8 + e:t * 8 + e + 1])
        ps_d5 = ps_big.tile([128, 1024], F32, tag="pbig", name="ps_d5")
        for h in range(2):
            nc.tensor.matmul(ps_d5[0:H, ts(h, 512)], k5t[:, ts(e, H)],
                             memt[:, ts(h, 512)], start=True, stop=True)
        d5sb = e1pool.tile([5, 1024], F32, tag="d5sb")
        nc.scalar.copy(d5sb[:], ps_d5[0:H, :])
        nc.scalar.dma_start(d40[e * H:(e + 1) * H, :], d5sb[:])

    isc = gpool.tile([128, 64], F32, tag="isc")
    nc.scalar.activation(isc[:], mncol[:], AF.Sqrt, bias=epsv[:, :])
    nc.vector.reciprocal(isc[:], isc[:])
    imn8 = gpool.tile([8, 1024], F32, tag="imn8")
    for t in range(T):
        p6 = ps_tr.tile([8, 128], F32, tag="tr", name="p6")
        nc.tensor.transpose(p6[:], isc[:, ts(t, 8)], ident[:, :])
        nc.scalar.copy(imn8[:, ts(t, 128)], p6[:])
    ikn40 = gpool.tile([40, 1], F32, tag="ikn40")
    nc.vector.reciprocal(ikn40[:], kn40[:])
    iknsp40 = gpool.tile([40, 1], F32, tag="iknsp40")
    nc.vector.tensor_mul(iknsp40[:], ikn40[:], sp40[:])
    ps_imn40 = ps_big.tile([128, 1024], F32, tag="pbig", name="ps_imn40")
    for h in range(2):
        nc.tensor.matmul(ps_imn40[0:40, ts(h, 512)], selM[:, :], imn8[:, ts(h, 512)],
                         start=True, stop=True)
    z40 = tpool.tile([40, 1024], F32, tag="tmp40", name="z40")
    nc.scalar.mul(z40[:], ps_imn40[0:40, :], iknsp40[:])
    sharp40 = tpool.tile([40, 1024], F32, tag="tmp40", name="sharp40")
    nc.vector.tensor_mul(sharp40[:], d40[:], z40[:])

    mx40 = gpool.tile([40, 1], F32, tag="mx40")
    nc.vector.reduce_max(mx40[:], sharp40[:], axis=AX.X)
    negmx = gpool.tile([40, 1], F32, tag="negmx")
    nc.scalar.mul(negmx[:], mx40[:], -1.0)
    ex40 = tpool.tile([40, 1024], F32, tag="tmp40", name="ex40")
    sumex = gpool.tile([40, 1], F32, tag="sumex")
    nc.scalar.activation(ex40[:], sharp40[:], AF.Exp, bias=negmx[:], accum_out=sumex[:])
    inv40 = gpool.tile([40, 1], F32, tag="inv40")
    nc.vector.reciprocal(inv40[:], sumex[:])
    sm40 = gpool.tile([40, 1024], F32, tag="sm40")
    nc.scalar.mul(sm40[:], ex40[:], inv40[:])
    for e in range(E):
        nc.scalar.dma_start(orw_d[e, :, :], sm40[e * H:e * H + R, :])

    # ---- per-element pipeline: allocation -> write weights -> link ----
    dbg = {}
    if os.environ.get("K_DEBUG") == "1":
        dbg["wn"] = nc.dram_tensor("o_dbg_wn", [E, 128, N], F32, kind="ExternalOutput")
        dbg["pb"] = nc.dram_tensor("o_dbg_pb", [E, 128, N], F32, kind="ExternalOutput")
        dbg["wc"] = nc.dram_tensor("o_dbg_wc", [E, 128, 8], F32, kind="ExternalOutput")
        dbg["ac"] = nc.dram_tensor("o_dbg_ac", [E, 128, 8], F32, kind="ExternalOutput")
    for e in range(E):
        ua_e = erow.tile([1, 1024], F32, tag="erow", name="ua_e")
        nc.sync.dma_start(ua_e[:], ua8[e:e + 1, :])
        nonu_e = erow.tile([1, 1024], F32, tag="erow", name="nonu_e")
        nc.sync.dma_start(nonu_e[:], nonu8[e:e + 1, :])
        wc_e = erow.tile([1, 1024], F32, tag="erow", name="wc_e")
        nc.sync.dma_start(wc_e[:], sm40[e * H + R:e * H + H, :])
        pp_e = erow.tile([1, 1024], F32, tag="erow", name="pp_e")
        nc.sync.dma_start(pp_e[:], pp_d[e:e + 1, 0, :])
        ag_e = erow.tile([1, 1], F32, tag="gates", name="ag_e")
        nc.sync.dma_start(ag_e[:], ag_d[e:e + 1, :])
        wg_e = erow.tile([1, 1], F32, tag="gates", name="wg_e")
        nc.sync.dma_start(wg_e[:], wg_d[e:e + 1, :])

        # exact broadcast of ua_e over partitions: K=2 matmul with 0.5+0.5
        # weights on a doubled row (0.5*x + 0.5*x == x exactly in fp32)
        ua2 = epool.tile([2, 1024], F32, tag="stk2", name="ua2")
        nc.sync.dma_start(ua2[0:1, :], ua8[e:e + 1, :])
        nc.sync.dma_start(ua2[1:2, :], ua8[e:e + 1, :])
        psum_ub = ps_big.tile([128, 1024], F32, tag="pbig", name="psum_ub")
        for h in range(2):
            nc.tensor.matmul(psum_ub[:, ts(h, 512)], half2[:, :],
                             ua2[:, ts(h, 512)], start=True, stop=True)
        ub_sb = epool.tile([128, 1024], F32, tag="ub_sb")
        nc.scalar.copy(ub_sb[:], psum_ub[:])

        ps_sr = [ps_acc.tile([3, 512], F32, tag="pacc", name=f"ps_sr{h}") for h in range(2)]
        for t in range(T):
            ct = ctpool.tile([128, 1024], BF16, tag="CT")
            c = t * 8 + e
            nc.vector.tensor_scalar(ct[:], ub_sb[:], ucol[:, c:c + 1],
                                    None, OP.is_gt)
            for h in range(2):
                nc.tensor.matmul(ps_sr[h][:, :], l3[:, 3 * c:3 * c + 3],
                                 ct[:, ts(h, 512)], start=(t == 0), stop=(t == T - 1))
        es_e = erow.tile([1, 1024], F32, tag="erow", name="es_e")
        sc3 = e1pool.tile([3, 1024], F32, tag="sc3")
        for h in range(2):
            nc.scalar.copy(sc3[:, ts(h, 512)], ps_sr[h][:, :])
        for h in range(2):
            ps_c = ps_acc.tile([1, 512], F32, tag="pacc", name=f"ps_c{h}")
            nc.tensor.matmul(ps_c[:, :], ones3[:, :], sc3[:, ts(h, 512)],
                             start=True, stop=True)
            nc.scalar.activation(es_e[:, ts(h, 512)], ps_c[:, :], AF.Exp)
        alloc_e = erow.tile([1, 1024], F32, tag="erow", name="alloc_e")
        nc.vector.tensor_mul(alloc_e[:], nonu_e[:], es_e[:])

        # write weights + precedence for this element
        onem_ag = erow.tile([1, 1], F32, tag="gates", name="onem_ag")
        nc.scalar.activation(onem_ag[:], ag_e[:], AF.Identity, bias=1.0, scale=-1.0)
        c1_e = erow.tile([1, 1], F32, tag="gates", name="c1_e")
        nc.vector.tensor_mul(c1_e[:], wg_e[:], ag_e[:])
        c2_e = erow.tile([1, 1], F32, tag="gates", name="c2_e")
        nc.vector.tensor_mul(c2_e[:], wg_e[:], onem_ag[:])
        t2_e = erow.tile([1, 1024], F32, tag="erow", name="t2_e")
        nc.scalar.mul(t2_e[:], wc_e[:], c2_e[:])
        w_e = erow.tile([1, 1024], F32, tag="erow", name="w_e")
        ws_e = erow.tile([1, 1], F32, tag="gates", name="ws_e")
        nc.vector.scalar_tensor_tensor(w_e[:], alloc_e[:], c1_e[:], t2_e[:],
                                       OP.mult, OP.add, accum_out=ws_e[:])
        nc.scalar.dma_start(oww_d[e:e + 1, 0, :], w_e[:])
        onem_ws = erow.tile([1, 1], F32, tag="gates", name="onem_ws")
        nc.scalar.activation(onem_ws[:], ws_e[:], AF.Identity, bias=1.0, scale=-1.0)
        prec_e = erow.tile([1, 1024], F32, tag="erow", name="prec_e")
        nc.vector.scalar_tensor_tensor(prec_e[:], pp_e[:], onem_ws[:], w_e[:],
                                       OP.mult, OP.add)
        nc.scalar.dma_start(op_d[e:e + 1, 0, :], prec_e[:])

        # link broadcasts (K=2 exact halves) and w columns
        w2 = epool.tile([2, 1024], F32, tag="stk2", name="w2")
        nc.sync.dma_start(w2[0:1, :], w_e[:])
        nc.sync.dma_start(w2[1:2, :], w_e[:])
        psum_wb = ps_big.tile([128, 1024], F32, tag="pbig", name="psum_wb")
        for h in range(2):
            nc.tensor.matmul(psum_wb[:, ts(h, 512)], half2[:, :],
                             w2[:, ts(h, 512)], start=True, stop=True)
        wn_e = epool.tile([128, 1024], F32, tag="wn_e")
        nc.scalar.mul(wn_e[:], psum_wb[:], -1.0)
        pp2 = epool.tile([2, 1024], F32, tag="stk2", name="pp2")
        nc.sync.dma_start(pp2[0:1, :], pp_e[:])
        nc.sync.dma_start(pp2[1:2, :], pp_e[:])
        psum_pb = ps_big.tile([128, 1024], F32, tag="pbig", name="psum_pb")
        for h in range(2):
            nc.tensor.matmul(psum_pb[:, ts(h, 512)], half2[:, :],
                             pp2[:, ts(h, 512)], start=True, stop=True)
        pb_e = epool.tile([128, 1024], F32, tag="pb_e")
        nc.scalar.copy(pb_e[:], psum_pb[:])

        # w columns by transposing chunks of the (negated) broadcast: every
        # column of transpose(wn_e[:, chunk t]) holds -w[t*128+p] on partitions
        acol_e = epool.tile([128, 8], F32, tag="acol_e")
        wcol_e = epool.tile([128, 8], F32, tag="wcol_e")
        for t in range(T):
            ps_wc = ps_tr.tile([128, 128], F32, tag="tr", name="ps_wc")
            nc.tensor.transpose(ps_wc[:], wn_e[:, ts(t, 128)], ident[:, :])
            nc.scalar.activation(acol_e[:, t:t + 1], ps_wc[:, 0:1], AF.Identity,
                                 bias=1.0)
            nc.scalar.mul(wcol_e[:, t:t + 1], ps_wc[:, 0:1], -1.0)

        if dbg:
            nc.sync.dma_start(dbg["wn"][e], wn_e[:])
            nc.sync.dma_start(dbg["pb"][e], pb_e[:])
            nc.sync.dma_start(dbg["wc"][e], wcol_e[:])
            nc.sync.dma_start(dbg["ac"][e], acol_e[:])
        for t in range(T):
            lt = lpool.tile([128, 1024], F32, tag="L")
            nc.sync.dma_start(lt[:], pl_d[e, 0, ts(t, 128), :])
            ot = opool.tile([128, 1024], F32, tag="O")
            a_sc = acol_e[:, t:t + 1]
            w_sc = wcol_e[:, t:t + 1]
            nc.vector.scalar_tensor_tensor(ot[:], wn_e[:], a_sc, lt[:],
                                           OP.add, OP.mult)
            nc.vector.scalar_tensor_tensor(ot[:], pb_e[:], w_sc, ot[:],
                                           OP.mult, OP.add)
            nc.gpsimd.tensor_mul(ot[:, ts(t, 128)], ot[:, ts(t, 128)], dmask[:])
            nc.sync.dma_start(ol_d[e, 0, ts(t, 128), :], ot[:])
        if os.environ.get("K_ELT_BARRIER") == "1":
            tc.strict_bb_all_engine_barrier()

    ctx.close()


def _build():
    global _NC
    if _NC is not None:
        return _NC
    nc = bacc.Bacc("TRN2", target_bir_lowering=False, debug=False, num_devices=NCORES)
    mem_d = nc.dram_tensor("memory", [E, N, W], F32, kind="ExternalInput")
    rk_d = nc.dram_tensor("read_keys", [E, R, W], F32, kind="ExternalInput")
    rs_d = nc.dram_tensor("read_strengths", [E, R], F32, kind="ExternalInput")
    wk_d = nc.dram_tensor("write_keys", [E, NW, W], F32, kind="ExternalInput")
    ws_d = nc.dram_tensor("write_strengths", [E, NW], F32, kind="ExternalInput")
    fg_d = nc.dram_tensor("free_gate", [E, R], F32, kind="ExternalInput")
    ag_d = nc.dram_tensor("alloc_gate", [E, NW], F32, kind="ExternalInput")
    wg_d = nc.dram_tensor("write_gate", [E, NW], F32, kind="ExternalInput")
    prw_d = nc.dram_tensor("prev_read_weights", [E, R, N], F32, kind="ExternalInput")
    pww_d = nc.dram_tensor("prev_write_weights", [E, NW, N], F32, kind="ExternalInput")
    pu_d = nc.dram_tensor("prev_usage", [E, N], F32, kind="ExternalInput")
    pl_d = nc.dram_tensor("prev_link", [E, NW, N, N], F32, kind="ExternalInput")
    pp_d = nc.dram_tensor("prev_precedence", [E, NW, N], F32, kind="ExternalInput")
    ident_d = nc.dram_tensor("c_ident", [128, 128], F32, kind="ExternalInput")
    dmask_d = nc.dram_tensor("c_dmask", [128, 128], F32, kind="ExternalInput")
    sel8_d = nc.dram_tensor("c_sel8", [8, 1024], F32, kind="ExternalInput")
    selE4_d = nc.dram_tensor("c_selE4", [32, 8], F32, kind="ExternalInput")
    selM_d = nc.dram_tensor("c_selM", [8, 40], F32, kind="ExternalInput")
    orw_d = nc.dram_tensor("o_read_weights", [E, R, N], F32, kind="ExternalOutput")
    oww_d = nc.dram_tensor("o_write_weights", [E, NW, N], F32, kind="ExternalOutput")
    ou_d = nc.dram_tensor("o_usage", [E, N], F32, kind="ExternalOutput")
    ol_d = nc.dram_tensor("o_link", [E, NW, N, N], F32, kind="ExternalOutput")
    op_d = nc.dram_tensor("o_precedence", [E, NW, N], F32, kind="ExternalOutput")
    io = (mem_d, rk_d, rs_d, wk_d, ws_d, fg_d, ag_d, wg_d, prw_d, pww_d, pu_d,
          pl_d, pp_d, ident_d, dmask_d, sel8_d, selE4_d, selM_d,
          orw_d, oww_d, ou_d, ol_d, op_d)
    with tile.TileContext(nc) as tc:
        _emit(nc, tc, io)
    nc.compile()
    _NC = nc
    return nc


def _consts():
    eye = np.eye(128, dtype=np.float32)
    return {
        "c_ident": eye,
        "c_dmask": (1.0 - eye).astype(np.float32),
        "c_sel8": np.repeat(np.eye(8, dtype=np.float32), 128, axis=1),
        "c_selE4": np.repeat(np.eye(8, dtype=np.float32), 4, axis=0),
        "c_selM": np.repeat(np.eye(8, dtype=np.float32), 5, axis=1),
    }


def kernel(memory, read_keys, read_strengths, write_keys, write_strengths,
           free_gate, alloc_gate, write_gate, prev_read_weights,
           prev_write_weights, prev_usage, prev_link, prev_precedence):
    global LAST_RESULTS
    nc = _build()
    full = {
        "memory": memory, "read_keys": read_keys,
        "read_strengths": read_strengths, "write_keys": write_keys,
        "write_strengths": write_strengths, "free_gate": free_gate,
        "alloc_gate": alloc_gate, "write_gate": write_gate,
        "prev_read_weights": prev_read_weights,
        "prev_write_weights": prev_write_weights, "prev_usage": prev_usage,
        "prev_link": prev_link, "prev_precedence": prev_precedence,
    }
    consts = _consts()
    in_maps = []
    for c in range(NCORES):
        m = {k: np.ascontiguousarray(np.asarray(v)[c * E:(c + 1) * E],
                                     dtype=np.float32)
             for k, v in full.items()}
        m.update(consts)
        in_maps.append(m)
    res = run_bass_kernel_spmd(nc, in_maps, core_ids=list(range(NCORES)),
                               trace=TRACE, **TRACE_KW)
    LAST_RESULTS = res
    outs = res.results
    read_weights = np.concatenate([outs[c]["o_read_weights"] for c in range(NCORES)], 0)
    write_weights = np.concatenate([outs[c]["o_write_weights"] for c in range(NCORES)], 0)
    usage = np.concatenate([outs[c]["o_usage"] for c in range(NCORES)], 0)
    link = np.concatenate([outs[c]["o_link"] for c in range(NCORES)], 0)
    precedence = np.concatenate([outs[c]["o_precedence"] for c in range(NCORES)], 0)
    return (read_weights, write_weights, usage, link, precedence)


# revision 39
# speedup vs baseline: 1.0031x; 1.0031x over previous
"""Trainium2 Bass kernel for one DNC memory-addressing timestep.

Contract: kernel(**inputs) takes the FULL (unsharded) numpy inputs of
reference.setup_inputs() and returns the full outputs
(read_weights, write_weights, usage, link, precedence) as numpy float32.

Sharding: pure data parallel over batch dim 0 across 8 NeuronCores
(8 batch elements per core, no cross-core communication).

Algorithm notes (per batch element, N=1024 slots, W=64 word, R=4 reads):
  - usage / precedence / write weights: exact elementwise math, batched
    across the 8 per-core elements on partitions 0..7 ("row layout"
    [8, 1024]) so every vector op costs one instruction per core.
  - allocation: the reference sorts usage; here we use the equivalent
    closed form alloc[i] = nonusage[i] * prod_{j: u_j < u_i} u_j
                        = nonusage[i] * exp(sum_j [u_j < u_i] * log u_j)
    computed with comparison tiles (DVE is_gt) contracted against
    log(u) columns on the TensorEngine.  (fp32 ties have ~0 probability
    and were validated against the reference to 3e-8 absmax.)
  - cosine content addressing: dot products via PE after on-chip
    transposes of memory; softmax batched over all 8*5 head-rows.
  - link update (the memory-bound bulk: 8MB/elt of HBM traffic) is two
    fused scalar_tensor_tensor ops per [128, 1024] tile:
        out = (a_i - w_j) * L         (a = 1 - w, per-partition scalar)
        out = (p_j * w_i) + out
    plus a [128,128] diagonal mask multiply, split DVE/GPSIMD.
"""

import os
import sys
from contextlib import ExitStack

for _p in ("/opt/trn_rl_repo", "/root/.axon_site/_ro/trn_rl_repo"):
    if os.path.isdir(_p) and _p not in sys.path:
        sys.path.append(_p)

import numpy as np

import concourse.bass as bass
import concourse.bacc as bacc
import concourse.mybir as mybir
import concourse.tile as tile
from concourse.bass_utils import run_bass_kernel_spmd

F32 = mybir.dt.float32
F32R = mybir.dt.float32r
BF16 = mybir.dt.bfloat16
AF = mybir.ActivationFunctionType
OP = mybir.AluOpType
AX = mybir.AxisListType
ts = bass.ts

EPS = 1e-5
B, N, W, R, NW = 64, 1024, 64, 4, 1
NCORES = 8
E = B // NCORES          # batch elements per core = 8
T = N // 128             # 128-row chunks per link matrix = 8
H = R + NW               # stacked heads (4 read + 1 write) = 5

# How many of the 8 link-update "first op" tiles per element run on
# GPSIMD instead of DVE (load balancing knob).
GPS_STT = int(os.environ.get("K_GPS_STT", "0"))
# Engine for the diagonal mask multiply: "pool" or "dve".
DIAG_ENG = os.environ.get("K_DIAG_ENG", "pool")

TRACE = False
TRACE_KW = {}
LAST_RESULTS = None

_NC = None


def _emit(nc, tc, io):
    (mem_d, rk_d, rs_d, wk_d, ws_d, fg_d, ag_d, wg_d, prw_d, pww_d, pu_d,
     pl_d, pp_d, ident_d, dmask_d, sel8_d, selE4_d, selM_d,
     orw_d, oww_d, ou_d, ol_d, op_d) = io

    ctx = ExitStack()
    cpool = ctx.enter_context(tc.tile_pool(name="consts", bufs=1))
    gpool = ctx.enter_context(tc.tile_pool(name="glob", bufs=1))
    tpool = ctx.enter_context(tc.tile_pool(name="tmp", bufs=5))
    epool = ctx.enter_context(tc.tile_pool(name="pere", bufs=2))
    e1pool = ctx.enter_context(tc.tile_pool(name="pere1", bufs=1))
    erow = ctx.enter_context(tc.tile_pool(name="erow", bufs=10))
    ctpool = ctx.enter_context(tc.tile_pool(name="ct", bufs=3))
    lpool = ctx.enter_context(tc.tile_pool(name="lin", bufs=8))
    opool = ctx.enter_context(tc.tile_pool(name="lout", bufs=4))
    ps_big = ctx.enter_context(tc.tile_pool(name="ps_big", bufs=1, space="PSUM"))
    ps_acc = ctx.enter_context(tc.tile_pool(name="ps_acc", bufs=2, space="PSUM"))
    ps_ub = ctx.enter_context(tc.tile_pool(name="ps_ub", bufs=1, space="PSUM"))
    ps_tr = ctx.enter_context(tc.tile_pool(name="ps_tr", bufs=2, space="PSUM"))

    # ---- constants ----
    ident = cpool.tile([128, 128], F32, tag="ident")
    selE4 = cpool.tile([32, 8], F32, tag="selE4")
    selM = cpool.tile([8, 40], F32, tag="selM")
    epsv = cpool.tile([128, 1], F32, tag="epsv")
    ones_row = cpool.tile([1, 128], F32, tag="ones_row")
    half2 = cpool.tile([2, 128], F32, tag="half2")
    ones3 = cpool.tile([3, 1], F32, tag="ones3")
    dmask = cpool.tile([128, 128], F32, tag="dmask")
    nc.sync.dma_start(dmask[:], dmask_d[:])
    nc.sync.dma_start(ident[:], ident_d[:])
    nc.sync.dma_start(selE4[:], selE4_d[:])
    nc.sync.dma_start(selM[:], selM_d[:])
    nc.vector.memset(epsv[:], EPS)
    nc.vector.memset(ones_row[:], 1.0)
    nc.vector.memset(half2[:], 0.5)
    nc.vector.memset(ones3[:], 1.0)

    # ---- small input loads ----
    prw_all = gpool.tile([32, 1024], F32, tag="prw")       # rows 4e+r
    fg_all = gpool.tile([32, 1], F32, tag="fg")
    pu8 = gpool.tile([8, 1024], F32, tag="pu8")
    pww8 = gpool.tile([8, 1024], F32, tag="pww8")
    K40 = gpool.tile([40, 64], F32, tag="K40")             # rows 5e+h
    st40 = gpool.tile([40, 1], F32, tag="st40")
    nc.sync.dma_start(prw_all[:], prw_d[:, :, :].rearrange("e r n -> (e r) n"))
    nc.sync.dma_start(fg_all[:, 0], fg_d[:, :].rearrange("e r -> (e r)"))
    nc.sync.dma_start(pu8[:], pu_d[:, :])
    nc.sync.dma_start(pww8[:], pww_d[:, 0, :])
    for e in range(E):
        nc.sync.dma_start(K40[e * H:e * H + R, :], rk_d[e, :, :])
        nc.sync.dma_start(K40[e * H + R:e * H + H, :], wk_d[e, :, :])
        nc.sync.dma_start(st40[e * H:e * H + R, 0], rs_d[e, :])
        nc.sync.dma_start(st40[e * H + R:e * H + H, 0], ws_d[e, :])

    # ---- usage (batched rows [8, 1024]) ----
    negfg = gpool.tile([32, 1], F32, tag="negfg")
    nc.scalar.mul(negfg[:], fg_all[:], -1.0)
    t_all = tpool.tile([32, 1024], F32, tag="tmp40", name="t_all")
    nc.scalar.activation(t_all[:], prw_all[:], AF.Identity, bias=1.0, scale=negfg[:])
    lt_all = tpool.tile([32, 1024], F32, tag="tmp40", name="lt_all")
    nc.scalar.activation(lt_all[:], t_all[:], AF.Ln)
    psum_phi = ps_big.tile([128, 1024], F32, tag="pbig", name="psum_phi")
    for h in range(2):
        nc.tensor.matmul(psum_phi[0:8, ts(h, 512)], selE4[:, :], lt_all[:, ts(h, 512)],
                         start=True, stop=True)
    phi8 = tpool.tile([8, 1024], F32, tag="tmp40", name="phi8")
    nc.scalar.activation(phi8[:], psum_phi[0:8, :], AF.Exp)

    onem_pu8 = tpool.tile([8, 1024], F32, tag="tmp40", name="onem_pu8")
    nc.scalar.activation(onem_pu8[:], pu8[:], AF.Identity, bias=1.0, scale=-1.0)
    onem_pww8 = tpool.tile([8, 1024], F32, tag="tmp40", name="onem_pww8")
    nc.scalar.activation(onem_pww8[:], pww8[:], AF.Identity, bias=1.0, scale=-1.0)
    mm2 = tpool.tile([8, 1024], F32, tag="tmp40", name="mm2")
    nc.vector.tensor_mul(mm2[:], onem_pu8[:], onem_pww8[:])
    u18 = tpool.tile([8, 1024], F32, tag="tmp40", name="u18")
    nc.scalar.activation(u18[:], mm2[:], AF.Identity, bias=1.0, scale=-1.0)
    usage8 = tpool.tile([8, 1024], F32, tag="tmp40", name="usage8")
    nc.vector.tensor_mul(usage8[:], u18[:], phi8[:])
    nc.scalar.dma_start(ou_d[:, :], usage8[:])

    ua8 = gpool.tile([8, 1024], F32, tag="ua8")
    nc.scalar.activation(ua8[:], usage8[:], AF.Identity, bias=epsv[0:8, :], scale=1.0 - EPS)
    lua8 = tpool.tile([8, 1024], F32, tag="tmp40", name="lua8")
    nc.scalar.activation(lua8[:], ua8[:], AF.Ln)
    nonu8 = gpool.tile([8, 1024], F32, tag="nonu8")
    nc.scalar.activation(nonu8[:], ua8[:], AF.Identity, bias=1.0, scale=-1.0)

    # column layouts: ucol for compares, lcol -> 3-way bf16 split for matmuls
    ucol = gpool.tile([128, 64], F32, tag="ucol")   # [:, t*8+e]
    lcol = gpool.tile([128, 64], F32, tag="lcol")
    for t in range(T):
        p1 = ps_tr.tile([128, 8], F32, tag="tr", name="p1")
        nc.tensor.transpose(p1[:], ua8[:, ts(t, 128)], ident[0:8, 0:8])
        nc.scalar.copy(ucol[:, ts(t, 8)], p1[:])
        p2 = ps_tr.tile([128, 8], F32, tag="tr", name="p2")
        nc.tensor.transpose(p2[:], lua8[:, ts(t, 128)], ident[0:8, 0:8])
        nc.scalar.copy(lcol[:, ts(t, 8)], p2[:])
    l3 = gpool.tile([128, 192], BF16, tag="l3")
    l3v = l3[:].rearrange("p (c k) -> p c k", k=3)
    hi_f = gpool.tile([128, 64], F32, tag="hi_f")
    r1 = gpool.tile([128, 64], F32, tag="r1")
    nc.scalar.copy(l3v[:, :, 0], lcol[:])
    nc.scalar.copy(hi_f[:], l3v[:, :, 0])
    nc.vector.tensor_sub(r1[:], lcol[:], hi_f[:])
    nc.scalar.copy(l3v[:, :, 1], r1[:])
    nc.scalar.copy(hi_f[:], l3v[:, :, 1])
    nc.vector.tensor_sub(r1[:], r1[:], hi_f[:])
    nc.scalar.copy(l3v[:, :, 2], r1[:])

    # ---- cosine content weights (independent of usage; fully batched) ----
    p_k = ps_tr.tile([64, 40], F32, tag="tr", name="p_k")
    nc.tensor.transpose(p_k[:], K40[:], ident[0:40, 0:40])
    k5t = gpool.tile([64, 40], F32, tag="k5t")
    nc.scalar.copy(k5t[:], p_k[:])
    sqk = gpool.tile([40, 64], F32, tag="sqk")
    nc.scalar.activation(sqk[:], K40[:], AF.Square)
    kn2 = gpool.tile([40, 1], F32, tag="kn2")
    nc.vector.reduce_sum(kn2[:], sqk[:], axis=AX.X)
    kn40 = gpool.tile([40, 1], F32, tag="kn40")
    nc.scalar.activation(kn40[:], kn2[:], AF.Sqrt, bias=epsv[0:40, :])
    e40 = gpool.tile([40, 1], F32, tag="e40")
    nc.scalar.activation(e40[:], st40[:], AF.Exp)
    sp40 = gpool.tile([40, 1], F32, tag="sp40")
    nc.scalar.activation(sp40[:], e40[:], AF.Ln, bias=1.0)

    mncol = gpool.tile([128, 64], F32, tag="mncol")   # [:, t*8+e] = ||mem row||^2
    d40 = gpool.tile([40, 1024], F32, tag="d40")
    for e in range(E):
        mem_e = epool.tile([128, 512], F32, tag="mem_e")
        nc.sync.dma_start(
            mem_e[:].rearrange("p (t w) -> p t w", w=W),
            mem_d[e].rearrange("(t p) w -> p t w", p=128),
        )
        memt = e1pool.tile([64, 1024], F32, tag="memt")
        sqd = epool.tile([128, 64], F32, tag="sqd")
        for t in range(T):
            p3 = ps_tr.tile([64, 128], F32, tag="tr", name="p3")
            nc.tensor.transpose(p3[:], mem_e[:, ts(t, W)], ident[:, :])
            nc.scalar.copy(memt[:, ts(t, 128)], p3[:])
            nc.scalar.activation(sqd[:], mem_e[:, ts(t, W)], AF.Square,
                                 accum_out=mncol[:, t * 8 + e:t * 8 + e + 1])
        ps_d5 = ps_big.tile([128, 1024], F32, tag="pbig", name="ps_d5")
        for h in range(2):
            nc.tensor.matmul(ps_d5[0:H, ts(h, 512)], k5t[:, ts(e, H)],
                             memt[:, ts(h, 512)], start=True, stop=True)
        d5sb = e1pool.tile([5, 1024], F32, tag="d5sb")
        nc.scalar.copy(d5sb[:], ps_d5[0:H, :])
        nc.scalar.dma_start(d40[e * H:(e + 1) * H, :], d5sb[:])

    isc = gpool.tile([128, 64], F32, tag="isc")
    nc.scalar.activation(isc[:], mncol[:], AF.Sqrt, bias=epsv[:, :])
    nc.vector.reciprocal(isc[:], isc[:])
    imn8 = gpool.tile([8, 1024], F32, tag="imn8")
    for t in range(T):
        p6 = ps_tr.tile([8, 128], F32, tag="tr", name="p6")
        nc.tensor.transpose(p6[:], isc[:, ts(t, 8)], ident[:, :])
        nc.scalar.copy(imn8[:, ts(t, 128)], p6[:])
    ikn40 = gpool.tile([40, 1], F32, tag="ikn40")
    nc.vector.reciprocal(ikn40[:], kn40[:])
    iknsp40 = gpool.tile([40, 1], F32, tag="iknsp40")
    nc.vector.tensor_mul(iknsp40[:], ikn40[:], sp40[:])
    ps_imn40 = ps_big.tile([128, 1024], F32, tag="pbig", name="ps_imn40")
    for h in range(2):
        nc.tensor.matmul(ps_imn40[0:40, ts(h, 512)], selM[:, :], imn8[:, ts(h, 512)],
                         start=True, stop=True)
    z40 = tpool.tile([40, 1024], F32, tag="tmp40", name="z40")
    nc.scalar.mul(z40[:], ps_imn40[0:40, :], iknsp40[:])
    sharp40 = tpool.tile([40, 1024], F32, tag="tmp40", name="sharp40")
    nc.vector.tensor_mul(sharp40[:], d40[:], z40[:])

    mx40 = gpool.tile([40, 1], F32, tag="mx40")
    nc.vector.reduce_max(mx40[:], sharp40[:], axis=AX.X)
    negmx = gpool.tile([40, 1], F32, tag="negmx")
    nc.scalar.mul(negmx[:], mx40[:], -1.0)
    ex40 = tpool.tile([40, 1024], F32, tag="tmp40", name="ex40")
    sumex = gpool.tile([40, 1], F32, tag="sumex")
    nc.scalar.activation(ex40[:], sharp40[:], AF.Exp, bias=negmx[:], accum_out=sumex[:])
    inv40 = gpool.tile([40, 1], F32, tag="inv40")
    nc.vector.reciprocal(inv40[:], sumex[:])
    sm40 = gpool.tile([40, 1024], F32, tag="sm40")
    nc.scalar.mul(sm40[:], ex40[:], inv40[:])
    for e in range(E):
        nc.scalar.dma_start(orw_d[e, :, :], sm40[e * H:e * H + R, :])

    # ---- per-element pipeline: allocation -> write weights -> link ----
    dbg = {}
    if os.environ.get("K_DEBUG") == "1":
        dbg["wn"] = nc.dram_tensor("o_dbg_wn", [E, 128, N], F32, kind="ExternalOutput")
        dbg["pb"] = nc.dram_tensor("o_dbg_pb", [E, 128, N], F32, kind="ExternalOutput")
        dbg["wc"] = nc.dram_tensor("o_dbg_wc", [E, 128, 8], F32, kind="ExternalOutput")
        dbg["ac"] = nc.dram_tensor("o_dbg_ac", [E, 128, 8], F32, kind="ExternalOutput")
    for e in range(E):
        ua_e = erow.tile([1, 1024], F32, tag="erow", name="ua_e")
        nc.sync.dma_start(ua_e[:], ua8[e:e + 1, :])
        nonu_e = erow.tile([1, 1024], F32, tag="erow", name="nonu_e")
        nc.sync.dma_start(nonu_e[:], nonu8[e:e + 1, :])
        wc_e = erow.tile([1, 1024], F32, tag="erow", name="wc_e")
        nc.sync.dma_start(wc_e[:], sm40[e * H + R:e * H + H, :])
        pp_e = erow.tile([1, 1024], F32, tag="erow", name="pp_e")
        nc.sync.dma_start(pp_e[:], pp_d[e:e + 1, 0, :])
        ag_e = erow.tile([1, 1], F32, tag="gates", name="ag_e")
        nc.sync.dma_start(ag_e[:], ag_d[e:e + 1, :])
        wg_e = erow.tile([1, 1], F32, tag="gates", name="wg_e")
        nc.sync.dma_start(wg_e[:], wg_d[e:e + 1, :])

        # exact broadcast of ua_e over partitions: K=2 matmul with 0.5+0.5
        # weights on a doubled row (0.5*x + 0.5*x == x exactly in fp32)
        ua2 = epool.tile([2, 1024], F32, tag="stk2", name="ua2")
        nc.sync.dma_start(ua2[0:1, :], ua8[e:e + 1, :])
        nc.sync.dma_start(ua2[1:2, :], ua8[e:e + 1, :])
        psum_ub = ps_ub.tile([128, 1024], F32, tag="pub", name="psum_ub")
        for h in range(2):
            nc.tensor.matmul(psum_ub[:, ts(h, 512)], half2[:, :],
                             ua2[:, ts(h, 512)], start=True, stop=True)
        ub_sb = epool.tile([128, 1024], F32, tag="ub_sb")
        nc.scalar.copy(ub_sb[:], psum_ub[:])

        ps_sr = [ps_acc.tile([3, 512], F32, tag="pacc", name=f"ps_sr{h}") for h in range(2)]
        for t in range(T):
            ct = ctpool.tile([128, 1024], BF16, tag="CT")
            c = t * 8 + e
            nc.vector.tensor_scalar(ct[:], ub_sb[:], ucol[:, c:c + 1],
                                    None, OP.is_gt)
            for h in range(2):
                nc.tensor.matmul(ps_sr[h][:, :], l3[:, 3 * c:3 * c + 3],
                                 ct[:, ts(h, 512)], start=(t == 0), stop=(t == T - 1))
        es_e = erow.tile([1, 1024], F32, tag="erow", name="es_e")
        sc3 = e1pool.tile([3, 1024], F32, tag="sc3")
        for h in range(2):
            nc.scalar.copy(sc3[:, ts(h, 512)], ps_sr[h][:, :])
        for h in range(2):
            ps_c = ps_tr.tile([1, 512], F32, tag="tr", name=f"ps_c{h}")
            nc.tensor.matmul(ps_c[:, :], ones3[:, :], sc3[:, ts(h, 512)],
                             start=True, stop=True)
            nc.scalar.activation(es_e[:, ts(h, 512)], ps_c[:, :], AF.Exp)
        alloc_e = erow.tile([1, 1024], F32, tag="erow", name="alloc_e")
        nc.vector.tensor_mul(alloc_e[:], nonu_e[:], es_e[:])

        # write weights + precedence for this element
        onem_ag = erow.tile([1, 1], F32, tag="gates", name="onem_ag")
        nc.scalar.activation(onem_ag[:], ag_e[:], AF.Identity, bias=1.0, scale=-1.0)
        c1_e = erow.tile([1, 1], F32, tag="gates", name="c1_e")
        nc.vector.tensor_mul(c1_e[:], wg_e[:], ag_e[:])
        c2_e = erow.tile([1, 1], F32, tag="gates", name="c2_e")
        nc.vector.tensor_mul(c2_e[:], wg_e[:], onem_ag[:])
        t2_e = erow.tile([1, 1024], F32, tag="erow", name="t2_e")
        nc.scalar.mul(t2_e[:], wc_e[:], c2_e[:])
        w_e = erow.tile([1, 1024], F32, tag="erow", name="w_e")
        ws_e = erow.tile([1, 1], F32, tag="gates", name="ws_e")
        nc.vector.scalar_tensor_tensor(w_e[:], alloc_e[:], c1_e[:], t2_e[:],
                                       OP.mult, OP.add, accum_out=ws_e[:])
        nc.scalar.dma_start(oww_d[e:e + 1, 0, :], w_e[:])
        onem_ws = erow.tile([1, 1], F32, tag="gates", name="onem_ws")
        nc.scalar.activation(onem_ws[:], ws_e[:], AF.Identity, bias=1.0, scale=-1.0)
        prec_e = erow.tile([1, 1024], F32, tag="erow", name="prec_e")
        nc.vector.scalar_tensor_tensor(prec_e[:], pp_e[:], onem_ws[:], w_e[:],
                                       OP.mult, OP.add)
        nc.scalar.dma_start(op_d[e:e + 1, 0, :], prec_e[:])

        # link broadcasts (K=2 exact halves) and w columns
        w2 = epool.tile([2, 1024], F32, tag="stk2", name="w2")
        nc.sync.dma_start(w2[0:1, :], w_e[:])
        nc.sync.dma_start(w2[1:2, :], w_e[:])
        psum_wb = ps_big.tile([128, 1024], F32, tag="pbig", name="psum_wb")
        for h in range(2):
            nc.tensor.matmul(psum_wb[:, ts(h, 512)], half2[:, :],
                             w2[:, ts(h, 512)], start=True, stop=True)
        wn_e = epool.tile([128, 1024], F32, tag="wn_e")
        nc.scalar.mul(wn_e[:], psum_wb[:], -1.0)
        pp2 = epool.tile([2, 1024], F32, tag="stk2", name="pp2")
        nc.sync.dma_start(pp2[0:1, :], pp_e[:])
        nc.sync.dma_start(pp2[1:2, :], pp_e[:])
        psum_pb = ps_big.tile([128, 1024], F32, tag="pbig", name="psum_pb")
        for h in range(2):
            nc.tensor.matmul(psum_pb[:, ts(h, 512)], half2[:, :],
                             pp2[:, ts(h, 512)], start=True, stop=True)
        pb_e = epool.tile([128, 1024], F32, tag="pb_e")
        nc.scalar.copy(pb_e[:], psum_pb[:])

        # w columns by transposing chunks of the (negated) broadcast: every
        # column of transpose(wn_e[:, chunk t]) holds -w[t*128+p] on partitions
        acol_e = epool.tile([128, 8], F32, tag="acol_e")
        wcol_e = epool.tile([128, 8], F32, tag="wcol_e")
        for t in range(T):
            ps_wc = ps_tr.tile([128, 128], F32, tag="tr", name="ps_wc")
            nc.tensor.transpose(ps_wc[:], wn_e[:, ts(t, 128)], ident[:, :])
            nc.scalar.activation(acol_e[:, t:t + 1], ps_wc[:, 0:1], AF.Identity,
                                 bias=1.0)
            nc.scalar.mul(wcol_e[:, t:t + 1], ps_wc[:, 0:1], -1.0)

        if dbg:
            nc.sync.dma_start(dbg["wn"][e], wn_e[:])
            nc.sync.dma_start(dbg["pb"][e], pb_e[:])
            nc.sync.dma_start(dbg["wc"][e], wcol_e[:])
            nc.sync.dma_start(dbg["ac"][e], acol_e[:])
        for t in range(T):
            lt = lpool.tile([128, 1024], F32, tag="L")
            nc.sync.dma_start(lt[:], pl_d[e, 0, ts(t, 128), :])
            ot = opool.tile([128, 1024], F32, tag="O")
            a_sc = acol_e[:, t:t + 1]
            w_sc = wcol_e[:, t:t + 1]
            nc.vector.scalar_tensor_tensor(ot[:], wn_e[:], a_sc, lt[:],
                                           OP.add, OP.mult)
            nc.vector.scalar_tensor_tensor(ot[:], pb_e[:], w_sc, ot[:],
                                           OP.mult, OP.add)
            nc.gpsimd.tensor_mul(ot[:, ts(t, 128)], ot[:, ts(t, 128)], dmask[:])
            nc.sync.dma_start(ol_d[e, 0, ts(t, 128), :], ot[:])
        if os.environ.get("K_ELT_BARRIER") == "1":
            tc.strict_bb_all_engine_barrier()

    ctx.close()


def _build():
    global _NC
    if _NC is not None:
        return _NC
    nc = bacc.Bacc("TRN2", target_bir_lowering=False, debug=False, num_devices=NCORES)
    mem_d = nc.dram_tensor("memory", [E, N, W], F32, kind="ExternalInput")
    rk_d = nc.dram_tensor("read_keys", [E, R, W], F32, kind="ExternalInput")
    rs_d = nc.dram_tensor("read_strengths", [E, R], F32, kind="ExternalInput")
    wk_d = nc.dram_tensor("write_keys", [E, NW, W], F32, kind="ExternalInput")
    ws_d = nc.dram_tensor("write_strengths", [E, NW], F32, kind="ExternalInput")
    fg_d = nc.dram_tensor("free_gate", [E, R], F32, kind="ExternalInput")
    ag_d = nc.dram_tensor("alloc_gate", [E, NW], F32, kind="ExternalInput")
    wg_d = nc.dram_tensor("write_gate", [E, NW], F32, kind="ExternalInput")
    prw_d = nc.dram_tensor("prev_read_weights", [E, R, N], F32, kind="ExternalInput")
    pww_d = nc.dram_tensor("prev_write_weights", [E, NW, N], F32, kind="ExternalInput")
    pu_d = nc.dram_tensor("prev_usage", [E, N], F32, kind="ExternalInput")
    pl_d = nc.dram_tensor("prev_link", [E, NW, N, N], F32, kind="ExternalInput")
    pp_d = nc.dram_tensor("prev_precedence", [E, NW, N], F32, kind="ExternalInput")
    ident_d = nc.dram_tensor("c_ident", [128, 128], F32, kind="ExternalInput")
    dmask_d = nc.dram_tensor("c_dmask", [128, 128], F32, kind="ExternalInput")
    sel8_d = nc.dram_tensor("c_sel8", [8, 1024], F32, kind="ExternalInput")
    selE4_d = nc.dram_tensor("c_selE4", [32, 8], F32, kind="ExternalInput")
    selM_d = nc.dram_tensor("c_selM", [8, 40], F32, kind="ExternalInput")
    orw_d = nc.dram_tensor("o_read_weights", [E, R, N], F32, kind="ExternalOutput")
    oww_d = nc.dram_tensor("o_write_weights", [E, NW, N], F32, kind="ExternalOutput")
    ou_d = nc.dram_tensor("o_usage", [E, N], F32, kind="ExternalOutput")
    ol_d = nc.dram_tensor("o_link", [E, NW, N, N], F32, kind="ExternalOutput")
    op_d = nc.dram_tensor("o_precedence", [E, NW, N], F32, kind="ExternalOutput")
    io = (mem_d, rk_d, rs_d, wk_d, ws_d, fg_d, ag_d, wg_d, prw_d, pww_d, pu_d,
          pl_d, pp_d, ident_d, dmask_d, sel8_d, selE4_d, selM_d,
          orw_d, oww_d, ou_d, ol_d, op_d)
    with tile.TileContext(nc) as tc:
        _emit(nc, tc, io)
    nc.compile()
    _NC = nc
    return nc


def _consts():
    eye = np.eye(128, dtype=np.float32)
    return {
        "c_ident": eye,
        "c_dmask": (1.0 - eye).astype(np.float32),
        "c_sel8": np.repeat(np.eye(8, dtype=np.float32), 128, axis=1),
        "c_selE4": np.repeat(np.eye(8, dtype=np.float32), 4, axis=0),
        "c_selM": np.repeat(np.eye(8, dtype=np.float32), 5, axis=1),
    }


def kernel(memory, read_keys, read_strengths, write_keys, write_strengths,
           free_gate, alloc_gate, write_gate, prev_read_weights,
           prev_write_weights, prev_usage, prev_link, prev_precedence):
    global LAST_RESULTS
    nc = _build()
    full = {
        "memory": memory, "read_keys": read_keys,
        "read_strengths": read_strengths, "write_keys": write_keys,
        "write_strengths": write_strengths, "free_gate": free_gate,
        "alloc_gate": alloc_gate, "write_gate": write_gate,
        "prev_read_weights": prev_read_weights,
        "prev_write_weights": prev_write_weights, "prev_usage": prev_usage,
        "prev_link": prev_link, "prev_precedence": prev_precedence,
    }
    consts = _consts()
    in_maps = []
    for c in range(NCORES):
        m = {k: np.ascontiguousarray(np.asarray(v)[c * E:(c + 1) * E],
                                     dtype=np.float32)
             for k, v in full.items()}
        m.update(consts)
        in_maps.append(m)
    res = run_bass_kernel_spmd(nc, in_maps, core_ids=list(range(NCORES)),
                               trace=TRACE, **TRACE_KW)
    LAST_RESULTS = res
    outs = res.results
    read_weights = np.concatenate([outs[c]["o_read_weights"] for c in range(NCORES)], 0)
    write_weights = np.concatenate([outs[c]["o_write_weights"] for c in range(NCORES)], 0)
    usage = np.concatenate([outs[c]["o_usage"] for c in range(NCORES)], 0)
    link = np.concatenate([outs[c]["o_link"] for c in range(NCORES)], 0)
    precedence = np.concatenate([outs[c]["o_precedence"] for c in range(NCORES)], 0)
    return (read_weights, write_weights, usage, link, precedence)
